# revision 7
# baseline (speedup 1.0000x reference)
"""Trainium2 Bass kernel for nn_ClusteringLayer (vq_codebook).

Computes, for z:[N,D] f32 and cluster_layer:[K,D] f32 (N=65536, K=256, D=512):
    d2   = ||z_n - c_k||^2
    q    = (1 / (1 + d2)) row-normalized          (ALPHA = 1 -> exponent 1)

Strategy (8 NeuronCores, data-parallel over N):
  host pre-transposes each z shard to zT [D, Nc] (marshaling only; all FLOPs
  on device).  Device processes ST=8 row-tiles (1024 rows, 2 MB) per DMA
  slab -- large DMAs amortize SWDGE descriptor generation and store issue --
  with compute batched in pairs of tiles:

  per slab (8 tiles, one SWDGE cast-DMA f32->bf16):
    PE   per pair: 8 matmuls  psum_m += zT_b^T @ (-2 c^T_b)       [128,512]
                   8 matmuls  psum_g += zT_b^T @ zT_b (Gram diag) [128,256]
                   1 transpose z2pair [128,2] -> psum rows
                   1 augment matmul psum_m += [z2_A; z2_B; ones]^T @
                     [sel_A; sel_B; (1+||c_k||^2)]  (completes the denom)
    DVE  per pair: mask-mul + 3D-reduce -> z2pair; recip_approx on the
                   whole pair straight from PSUM; 1/s; per-tile
                   tensor_scalar normalize
    ACT  per pair: copy z2 rows psum->SBUF (bf16 cast); per-tile
                   copy+accum_out row sums
    one batched store DMA per slab.

  constants (c^T * -2, (1+||c||^2) folded into the select mask via a
  one-time SBUF->SBUF DMA) built on device once from cluster_layer.

  Output is written bf16 (halves store traffic; host upcasts) -- adds
  ~4e-3 max rel err from output quantization, well inside tolerance.

  HW notes (this axon/TRN2 runtime): Kc=1 matmuls hang -> pad to 32;
  tensor_tensor_reduce crashes -> unfused mul+reduce; ACT Reciprocal
  banned -> DVE reciprocal_approx_fast (~51 ULP).  Slab sizes shrink at
  the end of the schedule (shorter pipeline drain).  Cost model predicts
  ~67us/core, DVE 83% / PE 77% / ACT 74% busy, near the ~59us HBM
  roofline for 21 MB/core of traffic.
"""

import os
import sys
import numpy as np

for _p in ("/opt/trn_rl_repo", "/opt/pypackages"):
    if _p not in sys.path:
        sys.path.append(_p)

import ml_dtypes  # noqa: E402

import concourse.bass as bass  # noqa: E402
from concourse import bacc, mybir, tile  # noqa: E402
from concourse import bass_utils  # noqa: E402

F32 = mybir.dt.float32
BF16 = mybir.dt.bfloat16
AFT = mybir.ActivationFunctionType

N_CORES = 8
N, D, K = 65536, 512, 256
NC = N // N_CORES          # rows per core
P = 128                    # partitions
DB = D // P                # 4 d-blocks
NT = NC // P               # 64 tiles per core
ST = 8                     # tiles per slab (one load DMA / store DMA)
AUGK = 32                  # contraction pad for augment matmuls (Kc=1 hangs)

# --- tuning flags -----------------------------------------------------------
OUT_BF16 = True            # device writes q in bf16 (host upcasts)
TTR_FUSED = False          # fused diag extract (tensor_tensor_reduce)
ZPOOL_BUFS = 4
POST_BUFS = 5


def slab_schedule(nt):
    """Slab sizes: ST in the middle, shrinking at the end (shorter pipeline
    drain).  All sizes even (compute runs on pairs of tiles)."""
    rem = nt
    tail = []
    for r in [4, 2, 2]:
        if rem - r >= 0:
            tail.append(r)
            rem -= r
    mid = [ST] * (rem // ST)
    rem -= ST * (rem // ST)
    if rem:
        mid.append(rem)
    # tail slabs shrink toward the end: [.., 4, 2, 2]
    out = mid + sorted(tail, reverse=True)
    assert sum(out) == nt and all(x % 2 == 0 for x in out), out
    return out


def emit(tc, Q, ZT, CL, IDENT, ZSEL, nt=NT):
    """Emit kernel body. Q:[nt*128,K] out; ZT:[D,nt*128]; CL:[K,D];
    IDENT:[128,256] f32 = [I | I]."""
    nc = tc.nc
    out_dt = Q.dtype
    assert nt % 2 == 0
    schedule = slab_schedule(nt)

    ZTv = ZT.rearrange("(b p) n -> p b n", p=P)       # d = b*128 + p
    Qv = Q.rearrange("(t p) k -> p t k", p=P)         # row = t*128 + p

    with (
        tc.tile_pool(name="const", bufs=1) as const,
        tc.tile_pool(name="cpsum", bufs=1, space="PSUM") as cpsum,
        tc.tile_pool(name="zslab", bufs=ZPOOL_BUFS) as zpool,
        tc.tile_pool(name="psum_m", bufs=3, space="PSUM") as pm_pool,
        tc.tile_pool(name="psum_g", bufs=2, space="PSUM") as pg_pool,
        tc.tile_pool(name="psum_t", bufs=2, space="PSUM") as pt_pool,
        tc.tile_pool(name="post", bufs=POST_BUFS) as post,
        tc.tile_pool(name="small", bufs=POST_BUFS * 2) as small,
    ):
        # ---------------- constant prep (one-time) ----------------
        ident_sb = const.tile([P, 2 * P], F32)        # [I | I]
        nc.sync.dma_start(ident_sb[:], IDENT[:])
        ident1 = ident_sb[:, 0:P]                     # plain I for transposes

        cnat = const.tile([P, 2, D], F32)             # c rows [0:128],[128:256]
        nc.sync.dma_start(cnat[:, 0, :], CL[0:P, :])
        nc.sync.dma_start(cnat[:, 1, :], CL[P:K, :])

        # cT (scaled by -2), bf16, laid out [p, b, k]
        ctm2 = const.tile([P, DB, K], BF16)
        for b in range(DB):
            pc = cpsum.tile([P, K], F32, tag="cps")
            for kb in range(2):
                nc.tensor.transpose(
                    pc[:, kb * P:(kb + 1) * P],
                    cnat[:, kb, b * P:(b + 1) * P],
                    ident1,
                )
            nc.scalar.mul(ctm2[:, b, :], pc[:], -2.0)

        # c2 = sum_d c_k^2 (ctm2^2 = 4 c^2 -> scale 0.25)
        csq = const.tile([P, DB, K], BF16)
        for b in range(DB):
            nc.vector.tensor_mul(csq[:, b, :], ctm2[:, b, :], ctm2[:, b, :])
        ones_col = const.tile([P, AUGK], BF16)
        nc.vector.memset(ones_col[:], 1.0)
        c2p = cpsum.tile([AUGK, K], F32, tag="cps")
        for b in range(DB):
            nc.tensor.matmul(
                c2p[:], ones_col[:], csq[:, b, :],
                start=(b == 0), stop=(b == DB - 1),
            )
        # c2rep row0 = (1 + c2) | (1 + c2)  (for a pair of tiles)
        c2rep = const.tile([AUGK, 2 * K], BF16)
        nc.vector.memset(c2rep[:], 0.0)
        for h in range(2):
            nc.scalar.activation(
                c2rep[0:1, h * K:(h + 1) * K], c2p[0:1, :], AFT.Relu,
                bias=1.0, scale=0.25,
            )
        # ones row for the c2 augment
        ones_row = const.tile([AUGK, P], BF16)
        nc.vector.memset(ones_row[:], 0.0)
        nc.vector.memset(ones_row[0:1, :], 1.0)
        # select mask (host input): row0 -> first tile of pair, row1 -> 2nd
        zsel = const.tile([AUGK, 2 * K], BF16)
        nc.sync.dma_start(zsel[:], ZSEL[:])
        # (1+c2) row rides the same augment matmul: zsel row2 <- c2rep row0
        nc.sync.dma_start(zsel[2:3, :], c2rep[0:1, :])
        # z2 row staging (rows 3+ stay zero; halves alternate by pair parity;
        # row2 = ones so zsel row2 contributes (1+c2) to every output row)
        z2sb = const.tile([AUGK, 2, P], BF16)
        nc.vector.memset(z2sb[:], 0.0)
        for par in range(2):
            nc.sync.dma_start(z2sb[2:3, par, :], ones_row[0:1, :])

        # ---------------- main loop over slabs ----------------
        tile0 = 0
        pair_ctr = 0
        for st_i in schedule:
            slab = zpool.tile([P, DB, st_i * P], BF16, tag="slab")
            # split slab loads: Tile tracks sub-tile regions, so the first
            # pairs' matmuls start as soon as their part lands.  1 MB halves
            # stay on the efficient part of the DMA-size curve; the very
            # first slab uses quarters (fill latency beats peak efficiency
            # there, and it's one slab out of ten).
            nparts = 4 if tile0 == 0 else 2
            pw = st_i * P // nparts
            for qq in range(nparts):
                nc.gpsimd.dma_start(
                    slab[:, :, qq * pw:(qq + 1) * pw],
                    ZTv[:, :, tile0 * P + qq * pw:tile0 * P + (qq + 1) * pw])

            qout = post.tile([P, st_i, K], out_dt, tag="qout")
            spair = small.tile([P, st_i], F32, tag="s")
            sinv = small.tile([P, st_i], F32, tag="sinv")

            for half in range(st_i // 2):             # pair of tiles
                par = pair_ctr % 2
                pair_ctr += 1
                psum_m = pm_pool.tile([P, 2 * K], F32, tag="pm")
                psum_g = pg_pool.tile([P, 2 * P], F32, tag="pg")
                for tt in range(2):
                    t = half * 2 + tt                 # tile within slab
                    zsl = slab[:, :, t * P:(t + 1) * P]
                    for b in range(DB):
                        nc.tensor.matmul(
                            psum_m[:, tt * K:(tt + 1) * K],
                            zsl[:, b, :], ctm2[:, b, :],
                            start=(tt == 0 and b == 0), stop=False,
                            skip_group_check=True,
                        )
                        nc.tensor.matmul(
                            psum_g[:, tt * P:(tt + 1) * P],
                            zsl[:, b, :], zsl[:, b, :],
                            start=(tt == 0 and b == 0),
                            stop=(tt == 1 and b == DB - 1),
                            skip_group_check=True,
                        )

                # z2 per tile of the pair: diag(psum_g).  z2pair is padded
                # to 32 cols (transpose with tiny stationary dims is risky);
                # cols 2..31 are garbage and never read downstream.
                scrap = post.tile([P, 2 * P], F32, tag="scrap")
                z2pair = small.tile([P, AUGK], F32, tag="z2")
                if TTR_FUSED:
                    for tt in range(2):
                        nc.vector.tensor_tensor_reduce(
                            out=scrap[:, tt * P:(tt + 1) * P],
                            in0=psum_g[:, tt * P:(tt + 1) * P],
                            in1=ident_sb[:, 0:P],
                            scale=1.0, scalar=0.0,
                            op0=mybir.AluOpType.mult,
                            op1=mybir.AluOpType.add,
                            accum_out=z2pair[:, tt:tt + 1],
                        )
                else:
                    nc.vector.tensor_mul(scrap[:], psum_g[:], ident_sb[:])
                    nc.vector.reduce_sum(
                        z2pair[:, 0:2],
                        scrap[:].rearrange("p (t n) -> p t n", t=2),
                        axis=mybir.AxisListType.X)

                # z2pair -> psum rows [32, 128] -> SBUF bf16 staging (rows 0:2)
                z2t = pt_pool.tile([AUGK, P], F32, tag="z2t")
                nc.tensor.transpose(z2t[:], z2pair[:], ident1)
                nc.scalar.copy(z2sb[0:2, par, :], z2t[0:2, :])

                # augment: += z2[n] (rows 0/1) and += (1+c2[k]) (row 2)
                nc.tensor.matmul(
                    psum_m[:], z2sb[:, par, :], zsel[:],
                    start=False, stop=True,
                    skip_group_check=True,
                )

                # q_un = 1/denom straight from PSUM (~51 ULP)
                qun = post.tile([P, 2 * K], F32, tag="qun")
                nc.vector.reciprocal_approx_fast(out=qun[:], in_=psum_m[:])

                # row sums via ACT copy+accum (per tile); in bf16-out mode
                # the copy also casts so the final scale runs at 4x
                qun2 = post.tile([P, 2 * K], out_dt, tag="qun2")
                for tt in range(2):
                    t = half * 2 + tt
                    nc.scalar.activation(
                        qun2[:, tt * K:(tt + 1) * K],
                        qun[:, tt * K:(tt + 1) * K], AFT.Copy,
                        accum_out=spair[:, t:t + 1],
                    )
                nc.vector.reciprocal(
                    sinv[:, half * 2:half * 2 + 2],
                    spair[:, half * 2:half * 2 + 2])
                for tt in range(2):
                    t = half * 2 + tt
                    nc.vector.tensor_scalar_mul(
                        qout[:, t, :], qun2[:, tt * K:(tt + 1) * K],
                        sinv[:, t:t + 1])

            nc.sync.dma_start(
                Qv[:, tile0:tile0 + st_i, :], qout[:])
            tile0 += st_i


def build_nc(nt=NT):
    nc = bacc.Bacc(
        "TRN2",
        target_bir_lowering=False,
        debug=False,
        enable_asserts=False,
    )
    out_dt = BF16 if OUT_BF16 else F32
    rows = nt * P
    ZT = nc.dram_tensor("zt", [D, rows], BF16, kind="ExternalInput").ap()
    CL = nc.dram_tensor("cl", [K, D], F32, kind="ExternalInput").ap()
    IDENT = nc.dram_tensor("ident", [P, 2 * P], F32, kind="ExternalInput").ap()
    ZSEL = nc.dram_tensor("zsel", [AUGK, 2 * K], BF16,
                          kind="ExternalInput").ap()
    Q = nc.dram_tensor("q", [rows, K], out_dt, kind="ExternalOutput").ap()

    with tile.TileContext(nc) as tc:
        emit(tc, Q, ZT, CL, IDENT, ZSEL, nt=nt)

    nc.compile()
    return nc


_CACHE = {}


def _get_nc():
    if "nc" not in _CACHE:
        _CACHE["nc"] = build_nc()
    return _CACHE["nc"]


def make_in_maps(z, cluster_layer):
    # z ships as bf16 [D, Nc]: the device matmuls consume bf16 either way
    # (the old path cast f32->bf16 during the load DMA), so pre-casting on
    # host is numerically equivalent and halves both the tunnel staging
    # bytes and the per-core HBM input traffic.
    zb = np.asarray(z, dtype=np.float32).astype(ml_dtypes.bfloat16)
    cl = np.ascontiguousarray(cluster_layer, dtype=np.float32)
    ident = np.tile(np.eye(P, dtype=np.float32), (1, 2))
    zsel = np.zeros((AUGK, 2 * K), dtype=ml_dtypes.bfloat16)
    zsel[0, 0:K] = 1.0
    zsel[1, K:2 * K] = 1.0
    in_maps = []
    for c in range(N_CORES):
        zt = np.ascontiguousarray(zb[c * NC:(c + 1) * NC].T)
        in_maps.append({"zt": zt, "cl": cl, "ident": ident, "zsel": zsel})
    return in_maps


class Runner:
    """Persistent 8-core PJRT runner (cached jit; callable repeatedly).

    Mirrors concourse.bass2jax.run_bass_via_pjrt's multi-core branch but
    keeps the jitted function alive so repeated calls skip retrace/compile.

    The axon tunnel to the remote TRN2 terminal has a ~80 ms round-trip
    and ~60 MB/s host<->device bandwidth; any per-call host staging
    dominates the actual device execution (~70 us).  So the steady-state
    call path keeps EVERYTHING device-resident: inputs are staged once
    (`stage_inputs`), and the donated output buffers are recycled -- call
    N's output array is handed back as call N+1's donated buffer (the
    kernel writes every element of q, so stale contents are harmless).
    One jitted sharded call == one tunnel round trip.
    """

    def __init__(self, nc):
        import jax
        from jax.experimental.shard_map import shard_map
        from jax.sharding import Mesh, PartitionSpec, NamedSharding
        from concourse import bass2jax

        bass2jax.install_neuronx_cc_hook()
        self.jax = jax
        self.nc = nc

        in_names, out_names, out_avals, zero_outs = [], [], [], []
        for alloc in nc.m.functions[0].allocations:
            if not isinstance(alloc, mybir.MemoryLocationSet):
                continue
            name = alloc.memorylocations[0].name
            if alloc.kind == "ExternalInput":
                in_names.append(name)
            elif alloc.kind == "ExternalOutput":
                out_names.append(name)
                shape = tuple(alloc.tensor_shape)
                dtype = mybir.dt.np(alloc.dtype)
                out_avals.append(jax.core.ShapedArray(shape, dtype))
                zero_outs.append(np.zeros(shape, dtype))
        assert nc.dbg_addr is None
        part_name = (nc.partition_id_tensor.name
                     if nc.partition_id_tensor else None)
        if part_name is not None and part_name in in_names:
            in_names.remove(part_name)
        self.in_names = list(in_names)
        self.out_names = out_names
        self.zero_outs = zero_outs
        n_params = len(in_names)
        n_outs = len(out_names)
        all_names = in_names + out_names
        if part_name is not None:
            all_names = all_names + [part_name]
        donate = tuple(range(n_params, n_params + n_outs))
        self.out_avals = out_avals

        def _body(*args):
            operands = list(args)
            if part_name is not None:
                operands.append(bass2jax.partition_id_tensor())
            outs = bass2jax._bass_exec_p.bind(
                *operands,
                out_avals=tuple(out_avals),
                in_names=tuple(all_names),
                out_names=tuple(out_names),
                lowering_input_output_aliases=(),
                sim_require_finite=False,
                sim_require_nnan=False,
                nc=nc,
            )
            return tuple(outs)

        devices = jax.devices()[:N_CORES]
        mesh = Mesh(np.asarray(devices), ("core",))
        in_specs = (PartitionSpec("core"),) * (n_params + n_outs)
        out_specs = (PartitionSpec("core"),) * n_outs
        self.sharding = NamedSharding(mesh, PartitionSpec("core"))
        self.fn = jax.jit(
            shard_map(_body, mesh=mesh, in_specs=in_specs,
                      out_specs=out_specs, check_rep=False),
            donate_argnums=donate, keep_unused=True,
        )
        self.in_dev = None           # device-staged inputs
        self.outbufs = None          # recycled donated output buffers
        self._jit_body = _body
        self._jit_kwargs = dict(mesh=mesh, in_specs=in_specs,
                                out_specs=out_specs)
        self._fast = None            # fast-dispatch Compiled (lazy)

    def concat_inputs(self, in_maps):
        return [
            np.concatenate([np.asarray(in_maps[c][n]) for c in range(N_CORES)],
                           axis=0)
            for n in self.in_names
        ]

    def stage_inputs(self, concat_in):
        """One-time host->device staging of inputs (sharded over cores)."""
        self.in_dev = [self.jax.device_put(a, self.sharding)
                       for a in concat_in]
        for a in self.in_dev:
            a.block_until_ready()

    def _ensure_outbufs(self):
        if self.outbufs is None:
            self.outbufs = [
                self.jax.device_put(
                    np.zeros((N_CORES * z.shape[0], *z.shape[1:]), z.dtype),
                    self.sharding)
                for z in self.zero_outs
            ]
            for o in self.outbufs:
                o.block_until_ready()

    def _ensure_fast(self):
        """AOT-compile the sharded body with the BassEffect suppressed
        (C++ fast-path dispatch, ~1 ms less host overhead per call).
        Falls back to the effectful jit on any failure."""
        if self._fast is not None:
            return
        try:
            from jax.experimental.shard_map import shard_map
            from concourse import bass2jax
            jax = self.jax
            example = list(self.in_dev) + list(self.outbufs)
            donate = tuple(range(len(self.in_names),
                                 len(self.in_names) + len(self.out_names)))

            def compile_fn():
                jfn = jax.jit(
                    shard_map(self._jit_body, check_rep=False,
                              **self._jit_kwargs),
                    donate_argnums=donate, keep_unused=True,
                )
                return jfn.lower(*example).compile()

            self._fast = bass2jax.fast_dispatch_compile(compile_fn)
        except Exception:
            self._fast = self.fn

    def step(self):
        """One kernel execution: single RPC, no host data movement.
        Donates the previous outputs as this call's output buffers."""
        self._ensure_outbufs()
        self._ensure_fast()
        outs = self._fast(*self.in_dev, *self.outbufs)
        self.outbufs = list(outs)
        return outs

    def run(self, in_maps):
        self.stage_inputs(self.concat_inputs(in_maps))
        out = self.step()
        q = np.asarray(out[0])
        return q


def _get_runner():
    if "runner" not in _CACHE:
        _CACHE["runner"] = Runner(_get_nc())
    return _CACHE["runner"]


def kernel(z, cluster_layer):
    runner = _get_runner()
    in_maps = make_in_maps(z, cluster_layer)
    q = runner.run(in_maps)
    return np.ascontiguousarray(q.astype(np.float32))


def ref_np(z, cl):
    d2 = np.maximum(
        (z * z).sum(1)[:, None] + (cl * cl).sum(1)[None, :]
        - 2.0 * (z @ cl.T), 0.0)
    qr = 1.0 / (1.0 + d2)
    qr /= qr.sum(1, keepdims=True)
    return qr


if __name__ == "__main__":
    rng = np.random.default_rng(0)
    z = rng.standard_normal((N, D), dtype=np.float32)
    cl = (rng.standard_normal((K, D), dtype=np.float32)
          * (2.0 / (K + D)) ** 0.5)
    q = kernel(z, cl)
    qr = ref_np(z, cl)
    err = np.abs(q - qr).max() / np.abs(qr).max()
    print("rel err:", err)



# revision 8
# speedup vs baseline: 1.0102x; 1.0102x over previous
"""Trainium2 Bass kernel for nn_ClusteringLayer (vq_codebook).

Computes, for z:[N,D] f32 and cluster_layer:[K,D] f32 (N=65536, K=256, D=512):
    d2   = ||z_n - c_k||^2
    q    = (1 / (1 + d2)) row-normalized          (ALPHA = 1 -> exponent 1)

Strategy (8 NeuronCores, data-parallel over N):
  host pre-transposes each z shard to zT [D, Nc] (marshaling only; all FLOPs
  on device).  Device processes ST=8 row-tiles (1024 rows, 2 MB) per DMA
  slab -- large DMAs amortize SWDGE descriptor generation and store issue --
  with compute batched in pairs of tiles:

  per slab (8 tiles, one SWDGE cast-DMA f32->bf16):
    PE   per pair: 8 matmuls  psum_m += zT_b^T @ (-2 c^T_b)       [128,512]
                   8 matmuls  psum_g += zT_b^T @ zT_b (Gram diag) [128,256]
                   1 transpose z2pair [128,2] -> psum rows
                   1 augment matmul psum_m += [z2_A; z2_B; ones]^T @
                     [sel_A; sel_B; (1+||c_k||^2)]  (completes the denom)
    DVE  per pair: mask-mul + 3D-reduce -> z2pair; recip_approx on the
                   whole pair straight from PSUM; 1/s; per-tile
                   tensor_scalar normalize
    ACT  per pair: copy z2 rows psum->SBUF (bf16 cast); per-tile
                   copy+accum_out row sums
    one batched store DMA per slab.

  constants (c^T * -2, (1+||c||^2) folded into the select mask via a
  one-time SBUF->SBUF DMA) built on device once from cluster_layer.

  Output is written bf16 (halves store traffic; host upcasts) -- adds
  ~4e-3 max rel err from output quantization, well inside tolerance.

  HW notes (this axon/TRN2 runtime): Kc=1 matmuls hang -> pad to 32;
  tensor_tensor_reduce crashes -> unfused mul+reduce; ACT Reciprocal
  banned -> DVE reciprocal_approx_fast (~51 ULP).  Slab sizes shrink at
  the end of the schedule (shorter pipeline drain).  Cost model predicts
  ~67us/core, DVE 83% / PE 77% / ACT 74% busy, near the ~59us HBM
  roofline for 21 MB/core of traffic.
"""

import os
import sys
import numpy as np

for _p in ("/opt/trn_rl_repo", "/opt/pypackages"):
    if _p not in sys.path:
        sys.path.append(_p)

import ml_dtypes  # noqa: E402

import concourse.bass as bass  # noqa: E402
from concourse import bacc, mybir, tile  # noqa: E402
from concourse import bass_utils  # noqa: E402

F32 = mybir.dt.float32
BF16 = mybir.dt.bfloat16
AFT = mybir.ActivationFunctionType

N_CORES = 8
N, D, K = 65536, 512, 256
NC = N // N_CORES          # rows per core
P = 128                    # partitions
DB = D // P                # 4 d-blocks
NT = NC // P               # 64 tiles per core
ST = 8                     # tiles per slab (one load DMA / store DMA)
AUGK = 32                  # contraction pad for augment matmuls (Kc=1 hangs)

# --- tuning flags -----------------------------------------------------------
OUT_BF16 = True            # device writes q in bf16 (host upcasts)
TTR_FUSED = False          # fused diag extract (tensor_tensor_reduce)
ZPOOL_BUFS = 4
POST_BUFS = 5


def slab_schedule(nt):
    """Slab sizes: ST in the middle, shrinking at the end (shorter pipeline
    drain).  All sizes even (compute runs on pairs of tiles)."""
    rem = nt
    tail = []
    for r in [4, 2, 2]:
        if rem - r >= 0:
            tail.append(r)
            rem -= r
    mid = [ST] * (rem // ST)
    rem -= ST * (rem // ST)
    if rem:
        mid.append(rem)
    # tail slabs shrink toward the end: [.., 4, 2, 2]
    out = mid + sorted(tail, reverse=True)
    assert sum(out) == nt and all(x % 2 == 0 for x in out), out
    return out


def emit(tc, Q, ZT, CL, IDENT, ZSEL, nt=NT):
    """Emit kernel body. Q:[nt*128,K] out; ZT:[D,nt*128]; CL:[K,D];
    IDENT:[128,256] f32 = [I | I]."""
    nc = tc.nc
    out_dt = Q.dtype
    assert nt % 2 == 0
    schedule = slab_schedule(nt)

    ZTv = ZT.rearrange("(b p) n -> p b n", p=P)       # d = b*128 + p
    Qv = Q.rearrange("(t p) k -> p t k", p=P)         # row = t*128 + p

    with (
        tc.tile_pool(name="const", bufs=1) as const,
        tc.tile_pool(name="cpsum", bufs=1, space="PSUM") as cpsum,
        tc.tile_pool(name="zslab", bufs=ZPOOL_BUFS) as zpool,
        tc.tile_pool(name="psum_m", bufs=3, space="PSUM") as pm_pool,
        tc.tile_pool(name="psum_g", bufs=2, space="PSUM") as pg_pool,
        tc.tile_pool(name="psum_t", bufs=2, space="PSUM") as pt_pool,
        tc.tile_pool(name="post", bufs=POST_BUFS) as post,
        tc.tile_pool(name="small", bufs=POST_BUFS * 2) as small,
    ):
        # ---------------- constant prep (one-time) ----------------
        ident_sb = const.tile([P, 2 * P], F32)        # [I | I]
        nc.sync.dma_start(ident_sb[:], IDENT[:])
        ident1 = ident_sb[:, 0:P]                     # plain I for transposes

        cnat = const.tile([P, 2, D], F32)             # c rows [0:128],[128:256]
        nc.sync.dma_start(cnat[:, 0, :], CL[0:P, :])
        nc.sync.dma_start(cnat[:, 1, :], CL[P:K, :])

        # cT (scaled by -2), bf16, laid out [p, b, k]
        ctm2 = const.tile([P, DB, K], BF16)
        for b in range(DB):
            pc = cpsum.tile([P, K], F32, tag="cps")
            for kb in range(2):
                nc.tensor.transpose(
                    pc[:, kb * P:(kb + 1) * P],
                    cnat[:, kb, b * P:(b + 1) * P],
                    ident1,
                )
            nc.scalar.mul(ctm2[:, b, :], pc[:], -2.0)

        # c2 = sum_d c_k^2 (ctm2^2 = 4 c^2 -> scale 0.25)
        csq = const.tile([P, DB, K], BF16)
        for b in range(DB):
            nc.vector.tensor_mul(csq[:, b, :], ctm2[:, b, :], ctm2[:, b, :])
        ones_col = const.tile([P, AUGK], BF16)
        nc.vector.memset(ones_col[:], 1.0)
        c2p = cpsum.tile([AUGK, K], F32, tag="cps")
        for b in range(DB):
            nc.tensor.matmul(
                c2p[:], ones_col[:], csq[:, b, :],
                start=(b == 0), stop=(b == DB - 1),
            )
        # c2rep row0 = (1 + c2) | (1 + c2)  (for a pair of tiles)
        c2rep = const.tile([AUGK, 2 * K], BF16)
        nc.vector.memset(c2rep[:], 0.0)
        for h in range(2):
            nc.scalar.activation(
                c2rep[0:1, h * K:(h + 1) * K], c2p[0:1, :], AFT.Relu,
                bias=1.0, scale=0.25,
            )
        # ones row for the c2 augment
        ones_row = const.tile([AUGK, P], BF16)
        nc.vector.memset(ones_row[:], 0.0)
        nc.vector.memset(ones_row[0:1, :], 1.0)
        # select mask (host input): row0 -> first tile of pair, row1 -> 2nd
        zsel = const.tile([AUGK, 2 * K], BF16)
        nc.sync.dma_start(zsel[:], ZSEL[:])
        # (1+c2) row rides the same augment matmul: zsel row2 <- c2rep row0
        nc.sync.dma_start(zsel[2:3, :], c2rep[0:1, :])
        # z2 row staging (rows 3+ stay zero; halves alternate by pair parity;
        # row2 = ones so zsel row2 contributes (1+c2) to every output row)
        z2sb = const.tile([AUGK, 2, P], BF16)
        nc.vector.memset(z2sb[:], 0.0)
        for par in range(2):
            nc.sync.dma_start(z2sb[2:3, par, :], ones_row[0:1, :])

        # ---------------- main loop over slabs ----------------
        tile0 = 0
        pair_ctr = 0
        for st_i in schedule:
            slab = zpool.tile([P, DB, st_i * P], BF16, tag="slab")
            # split slab loads: Tile tracks sub-tile regions, so the first
            # pairs' matmuls start as soon as their part lands.  1 MB halves
            # stay on the efficient part of the DMA-size curve; the very
            # first slab uses quarters (fill latency beats peak efficiency
            # there, and it's one slab out of ten).
            nparts = 4 if tile0 == 0 else 2
            pw = st_i * P // nparts
            for qq in range(nparts):
                nc.gpsimd.dma_start(
                    slab[:, :, qq * pw:(qq + 1) * pw],
                    ZTv[:, :, tile0 * P + qq * pw:tile0 * P + (qq + 1) * pw])

            qout = post.tile([P, st_i, K], out_dt, tag="qout")
            spair = small.tile([P, st_i], F32, tag="s")
            sinv = small.tile([P, st_i], F32, tag="sinv")

            for half in range(st_i // 2):             # pair of tiles
                par = pair_ctr % 2
                pair_ctr += 1
                psum_m = pm_pool.tile([P, 2 * K], F32, tag="pm")
                psum_g = pg_pool.tile([P, 2 * P], F32, tag="pg")
                for tt in range(2):
                    t = half * 2 + tt                 # tile within slab
                    zsl = slab[:, :, t * P:(t + 1) * P]
                    for b in range(DB):
                        nc.tensor.matmul(
                            psum_m[:, tt * K:(tt + 1) * K],
                            zsl[:, b, :], ctm2[:, b, :],
                            start=(tt == 0 and b == 0), stop=False,
                            skip_group_check=True,
                        )
                        nc.tensor.matmul(
                            psum_g[:, tt * P:(tt + 1) * P],
                            zsl[:, b, :], zsl[:, b, :],
                            start=(tt == 0 and b == 0),
                            stop=(tt == 1 and b == DB - 1),
                            skip_group_check=True,
                        )

                # z2 per tile of the pair: diag(psum_g).  z2pair is padded
                # to 32 cols (transpose with tiny stationary dims is risky);
                # cols 2..31 are garbage and never read downstream.
                scrap = post.tile([P, 2 * P], F32, tag="scrap")
                z2pair = small.tile([P, AUGK], F32, tag="z2")
                if TTR_FUSED:
                    for tt in range(2):
                        nc.vector.tensor_tensor_reduce(
                            out=scrap[:, tt * P:(tt + 1) * P],
                            in0=psum_g[:, tt * P:(tt + 1) * P],
                            in1=ident_sb[:, 0:P],
                            scale=1.0, scalar=0.0,
                            op0=mybir.AluOpType.mult,
                            op1=mybir.AluOpType.add,
                            accum_out=z2pair[:, tt:tt + 1],
                        )
                else:
                    nc.vector.tensor_mul(scrap[:], psum_g[:], ident_sb[:])
                    nc.vector.reduce_sum(
                        z2pair[:, 0:2],
                        scrap[:].rearrange("p (t n) -> p t n", t=2),
                        axis=mybir.AxisListType.X)

                # z2pair -> psum rows [32, 128] -> SBUF bf16 staging (rows 0:2)
                z2t = pt_pool.tile([AUGK, P], F32, tag="z2t")
                nc.tensor.transpose(z2t[:], z2pair[:], ident1)
                nc.scalar.copy(z2sb[0:2, par, :], z2t[0:2, :])

                # augment: += z2[n] (rows 0/1) and += (1+c2[k]) (row 2)
                nc.tensor.matmul(
                    psum_m[:], z2sb[:, par, :], zsel[:],
                    start=False, stop=True,
                    skip_group_check=True,
                )

                # q_un = 1/denom straight from PSUM (~51 ULP)
                qun = post.tile([P, 2 * K], F32, tag="qun")
                nc.vector.reciprocal_approx_fast(out=qun[:], in_=psum_m[:])

                # row sums via ACT copy+accum (per tile); in bf16-out mode
                # the copy also casts so the final scale runs at 4x
                qun2 = post.tile([P, 2 * K], out_dt, tag="qun2")
                for tt in range(2):
                    t = half * 2 + tt
                    nc.scalar.activation(
                        qun2[:, tt * K:(tt + 1) * K],
                        qun[:, tt * K:(tt + 1) * K], AFT.Copy,
                        accum_out=spair[:, t:t + 1],
                    )
                nc.vector.reciprocal(
                    sinv[:, half * 2:half * 2 + 2],
                    spair[:, half * 2:half * 2 + 2])
                for tt in range(2):
                    t = half * 2 + tt
                    nc.vector.tensor_scalar_mul(
                        qout[:, t, :], qun2[:, tt * K:(tt + 1) * K],
                        sinv[:, t:t + 1])

            nc.sync.dma_start(
                Qv[:, tile0:tile0 + st_i, :], qout[:])
            tile0 += st_i


def build_nc(nt=NT):
    nc = bacc.Bacc(
        "TRN2",
        target_bir_lowering=False,
        debug=False,
        enable_asserts=False,
    )
    out_dt = BF16 if OUT_BF16 else F32
    rows = nt * P
    ZT = nc.dram_tensor("zt", [D, rows], BF16, kind="ExternalInput").ap()
    CL = nc.dram_tensor("cl", [K, D], F32, kind="ExternalInput").ap()
    IDENT = nc.dram_tensor("ident", [P, 2 * P], F32, kind="ExternalInput").ap()
    ZSEL = nc.dram_tensor("zsel", [AUGK, 2 * K], BF16,
                          kind="ExternalInput").ap()
    Q = nc.dram_tensor("q", [rows, K], out_dt, kind="ExternalOutput").ap()

    with tile.TileContext(nc) as tc:
        emit(tc, Q, ZT, CL, IDENT, ZSEL, nt=nt)

    nc.compile()
    return nc


_CACHE = {}


def _get_nc():
    if "nc" not in _CACHE:
        _CACHE["nc"] = build_nc()
    return _CACHE["nc"]


def make_in_maps(z, cluster_layer):
    # z ships as bf16 [D, Nc]: the device matmuls consume bf16 either way
    # (the old path cast f32->bf16 during the load DMA), so pre-casting on
    # host is numerically equivalent and halves both the tunnel staging
    # bytes and the per-core HBM input traffic.
    zb = np.asarray(z, dtype=np.float32).astype(ml_dtypes.bfloat16)
    cl = np.ascontiguousarray(cluster_layer, dtype=np.float32)
    ident = np.tile(np.eye(P, dtype=np.float32), (1, 2))
    zsel = np.zeros((AUGK, 2 * K), dtype=ml_dtypes.bfloat16)
    zsel[0, 0:K] = 1.0
    zsel[1, K:2 * K] = 1.0
    in_maps = []
    for c in range(N_CORES):
        zt = np.ascontiguousarray(zb[c * NC:(c + 1) * NC].T)
        in_maps.append({"zt": zt, "cl": cl, "ident": ident, "zsel": zsel})
    return in_maps


class Runner:
    """Persistent 8-core PJRT runner (cached jit; callable repeatedly).

    Mirrors concourse.bass2jax.run_bass_via_pjrt's multi-core branch but
    keeps the jitted function alive so repeated calls skip retrace/compile.

    The axon tunnel to the remote TRN2 terminal has a ~80 ms round-trip
    and ~60 MB/s host<->device bandwidth; any per-call host staging
    dominates the actual device execution (~70 us).  So the steady-state
    call path keeps EVERYTHING device-resident: inputs are staged once
    (`stage_inputs`), and the donated output buffers are recycled -- call
    N's output array is handed back as call N+1's donated buffer (the
    kernel writes every element of q, so stale contents are harmless).
    One jitted sharded call == one tunnel round trip.
    """

    def __init__(self, nc):
        import jax
        from jax.experimental.shard_map import shard_map
        from jax.sharding import Mesh, PartitionSpec, NamedSharding
        from concourse import bass2jax

        bass2jax.install_neuronx_cc_hook()
        self.jax = jax
        self.nc = nc

        in_names, out_names, out_avals, zero_outs = [], [], [], []
        for alloc in nc.m.functions[0].allocations:
            if not isinstance(alloc, mybir.MemoryLocationSet):
                continue
            name = alloc.memorylocations[0].name
            if alloc.kind == "ExternalInput":
                in_names.append(name)
            elif alloc.kind == "ExternalOutput":
                out_names.append(name)
                shape = tuple(alloc.tensor_shape)
                dtype = mybir.dt.np(alloc.dtype)
                out_avals.append(jax.core.ShapedArray(shape, dtype))
                zero_outs.append(np.zeros(shape, dtype))
        assert nc.dbg_addr is None
        part_name = (nc.partition_id_tensor.name
                     if nc.partition_id_tensor else None)
        if part_name is not None and part_name in in_names:
            in_names.remove(part_name)
        self.in_names = list(in_names)
        self.out_names = out_names
        self.zero_outs = zero_outs
        n_params = len(in_names)
        n_outs = len(out_names)
        all_names = in_names + out_names
        if part_name is not None:
            all_names = all_names + [part_name]
        donate = tuple(range(n_params, n_params + n_outs))
        self.out_avals = out_avals

        def _body(*args):
            operands = list(args)
            if part_name is not None:
                operands.append(bass2jax.partition_id_tensor())
            outs = bass2jax._bass_exec_p.bind(
                *operands,
                out_avals=tuple(out_avals),
                in_names=tuple(all_names),
                out_names=tuple(out_names),
                lowering_input_output_aliases=(),
                sim_require_finite=False,
                sim_require_nnan=False,
                nc=nc,
            )
            return tuple(outs)

        devices = jax.devices()[:N_CORES]
        mesh = Mesh(np.asarray(devices), ("core",))
        in_specs = (PartitionSpec("core"),) * (n_params + n_outs)
        out_specs = (PartitionSpec("core"),) * n_outs
        self.sharding = NamedSharding(mesh, PartitionSpec("core"))
        self.fn = jax.jit(
            shard_map(_body, mesh=mesh, in_specs=in_specs,
                      out_specs=out_specs, check_rep=False),
            donate_argnums=donate, keep_unused=True,
        )
        self.in_dev = None           # device-staged inputs
        self.outbufs = None          # recycled donated output buffers
        self._jit_body = _body
        self._jit_kwargs = dict(mesh=mesh, in_specs=in_specs,
                                out_specs=out_specs)
        self._fast = None            # fast-dispatch Compiled (lazy)

    def concat_inputs(self, in_maps):
        return [
            np.concatenate([np.asarray(in_maps[c][n]) for c in range(N_CORES)],
                           axis=0)
            for n in self.in_names
        ]

    def stage_inputs(self, concat_in):
        """One-time host->device staging of inputs (sharded over cores)."""
        self.in_dev = [self.jax.device_put(a, self.sharding)
                       for a in concat_in]
        for a in self.in_dev:
            a.block_until_ready()

    def _ensure_outbufs(self):
        if self.outbufs is None:
            self.outbufs = [
                self.jax.device_put(
                    np.zeros((N_CORES * z.shape[0], *z.shape[1:]), z.dtype),
                    self.sharding)
                for z in self.zero_outs
            ]
            for o in self.outbufs:
                o.block_until_ready()

    def _ensure_fast(self):
        """AOT-compile the sharded body with the BassEffect suppressed
        (C++ fast-path dispatch, ~1 ms less host overhead per call).
        Falls back to the effectful jit on any failure."""
        if self._fast is not None:
            return
        try:
            from jax.experimental.shard_map import shard_map
            from concourse import bass2jax
            jax = self.jax
            example = list(self.in_dev) + list(self.outbufs)
            donate = tuple(range(len(self.in_names),
                                 len(self.in_names) + len(self.out_names)))

            def compile_fn():
                jfn = jax.jit(
                    shard_map(self._jit_body, check_rep=False,
                              **self._jit_kwargs),
                    donate_argnums=donate, keep_unused=True,
                )
                return jfn.lower(*example).compile()

            self._fast = bass2jax.fast_dispatch_compile(compile_fn)
        except Exception:
            self._fast = self.fn

    def reset_outbufs(self):
        self.outbufs = None

    def step(self):
        """One kernel execution: single RPC, no host data movement.
        Donates the previous outputs as this call's output buffers."""
        self._ensure_outbufs()
        self._ensure_fast()
        try:
            outs = self._fast(*self.in_dev, *self.outbufs)
        except Exception:
            # donated buffers are consumed even on failure; rebuild them
            # before the caller retries
            self.outbufs = None
            raise
        self.outbufs = list(outs)
        return outs

    def run(self, in_maps):
        self.stage_inputs(self.concat_inputs(in_maps))
        out = self.step()
        q = np.asarray(out[0])
        return q


def _get_runner():
    if "runner" not in _CACHE:
        _CACHE["runner"] = Runner(_get_nc())
    return _CACHE["runner"]


def kernel(z, cluster_layer):
    runner = _get_runner()
    in_maps = make_in_maps(z, cluster_layer)
    q = runner.run(in_maps)
    return np.ascontiguousarray(q.astype(np.float32))


def ref_np(z, cl):
    d2 = np.maximum(
        (z * z).sum(1)[:, None] + (cl * cl).sum(1)[None, :]
        - 2.0 * (z @ cl.T), 0.0)
    qr = 1.0 / (1.0 + d2)
    qr /= qr.sum(1, keepdims=True)
    return qr


if __name__ == "__main__":
    rng = np.random.default_rng(0)
    z = rng.standard_normal((N, D), dtype=np.float32)
    cl = (rng.standard_normal((K, D), dtype=np.float32)
          * (2.0 / (K + D)) ** 0.5)
    q = kernel(z, cl)
    qr = ref_np(z, cl)
    err = np.abs(q - qr).max() / np.abs(qr).max()
    print("rel err:", err)



# revision 9
# speedup vs baseline: 1.0184x; 1.0081x over previous
"""Trainium2 Bass kernel for nn_ClusteringLayer (vq_codebook).

Computes, for z:[N,D] f32 and cluster_layer:[K,D] f32 (N=65536, K=256, D=512):
    d2   = ||z_n - c_k||^2
    q    = (1 / (1 + d2)) row-normalized          (ALPHA = 1 -> exponent 1)

Strategy (8 NeuronCores, data-parallel over N):
  host pre-transposes each z shard to zT [D, Nc] (marshaling only; all FLOPs
  on device).  Device processes ST=8 row-tiles (1024 rows, 2 MB) per DMA
  slab -- large DMAs amortize SWDGE descriptor generation and store issue --
  with compute batched in pairs of tiles:

  per slab (8 tiles, one SWDGE cast-DMA f32->bf16):
    PE   per pair: 8 matmuls  psum_m += zT_b^T @ (-2 c^T_b)       [128,512]
                   8 matmuls  psum_g += zT_b^T @ zT_b (Gram diag) [128,256]
                   1 transpose z2pair [128,2] -> psum rows
                   1 augment matmul psum_m += [z2_A; z2_B; ones]^T @
                     [sel_A; sel_B; (1+||c_k||^2)]  (completes the denom)
    DVE  per pair: mask-mul + 3D-reduce -> z2pair; recip_approx on the
                   whole pair straight from PSUM; 1/s; per-tile
                   tensor_scalar normalize
    ACT  per pair: copy z2 rows psum->SBUF (bf16 cast); per-tile
                   copy+accum_out row sums
    one batched store DMA per slab.

  constants (c^T * -2, (1+||c||^2) folded into the select mask via a
  one-time SBUF->SBUF DMA) built on device once from cluster_layer.

  Output is written bf16 (halves store traffic; host upcasts) -- adds
  ~4e-3 max rel err from output quantization, well inside tolerance.

  HW notes (this axon/TRN2 runtime): Kc=1 matmuls hang -> pad to 32;
  tensor_tensor_reduce crashes -> unfused mul+reduce; ACT Reciprocal
  banned -> DVE reciprocal_approx_fast (~51 ULP).  Slab sizes shrink at
  the end of the schedule (shorter pipeline drain).  Cost model predicts
  ~67us/core, DVE 83% / PE 77% / ACT 74% busy, near the ~59us HBM
  roofline for 21 MB/core of traffic (z now ships bf16, halving load
  traffic to ~13 MB/core).

Dispatch path (the part that actually dominates wall time): the axon
tunnel to the remote TRN2 terminal has a ~70-90 ms round trip and only
~60 MB/s host<->device bandwidth, vs ~70 us of device execution.  The
original runner shipped 32 MB of zero-filled donated output buffers
host->device on EVERY call (~300-450 ms just in transfer).  The runner
below keeps all operands device-resident: inputs staged once, donated
output buffers recycled call-to-call (the kernel rewrites every element
of q), and the sharded executable AOT-compiled with the BassEffect
suppressed (C++ fast-path dispatch).  Steady-state call == exactly one
tunnel RPC == ~80 ms wall, which is the tunnel's round-trip floor
(an 8-byte device_put costs the same).
"""

import os
import sys
import numpy as np

for _p in ("/opt/trn_rl_repo", "/opt/pypackages"):
    if _p not in sys.path:
        sys.path.append(_p)

import ml_dtypes  # noqa: E402

import concourse.bass as bass  # noqa: E402
from concourse import bacc, mybir, tile  # noqa: E402
from concourse import bass_utils  # noqa: E402

F32 = mybir.dt.float32
BF16 = mybir.dt.bfloat16
AFT = mybir.ActivationFunctionType

N_CORES = 8
N, D, K = 65536, 512, 256
NC = N // N_CORES          # rows per core
P = 128                    # partitions
DB = D // P                # 4 d-blocks
NT = NC // P               # 64 tiles per core
ST = 8                     # tiles per slab (one load DMA / store DMA)
AUGK = 32                  # contraction pad for augment matmuls (Kc=1 hangs)

# --- tuning flags -----------------------------------------------------------
OUT_BF16 = True            # device writes q in bf16 (host upcasts)
TTR_FUSED = False          # fused diag extract (tensor_tensor_reduce)
ZPOOL_BUFS = 4
POST_BUFS = 5


def slab_schedule(nt):
    """Slab sizes: ST in the middle, shrinking at the end (shorter pipeline
    drain).  All sizes even (compute runs on pairs of tiles)."""
    rem = nt
    tail = []
    for r in [4, 2, 2]:
        if rem - r >= 0:
            tail.append(r)
            rem -= r
    mid = [ST] * (rem // ST)
    rem -= ST * (rem // ST)
    if rem:
        mid.append(rem)
    # tail slabs shrink toward the end: [.., 4, 2, 2]
    out = mid + sorted(tail, reverse=True)
    assert sum(out) == nt and all(x % 2 == 0 for x in out), out
    return out


def emit(tc, Q, ZT, CL, IDENT, ZSEL, nt=NT):
    """Emit kernel body. Q:[nt*128,K] out; ZT:[D,nt*128]; CL:[K,D];
    IDENT:[128,256] f32 = [I | I]."""
    nc = tc.nc
    out_dt = Q.dtype
    assert nt % 2 == 0
    schedule = slab_schedule(nt)

    ZTv = ZT.rearrange("(b p) n -> p b n", p=P)       # d = b*128 + p
    Qv = Q.rearrange("(t p) k -> p t k", p=P)         # row = t*128 + p

    with (
        tc.tile_pool(name="const", bufs=1) as const,
        tc.tile_pool(name="cpsum", bufs=1, space="PSUM") as cpsum,
        tc.tile_pool(name="zslab", bufs=ZPOOL_BUFS) as zpool,
        tc.tile_pool(name="psum_m", bufs=3, space="PSUM") as pm_pool,
        tc.tile_pool(name="psum_g", bufs=2, space="PSUM") as pg_pool,
        tc.tile_pool(name="psum_t", bufs=2, space="PSUM") as pt_pool,
        tc.tile_pool(name="post", bufs=POST_BUFS) as post,
        tc.tile_pool(name="small", bufs=POST_BUFS * 2) as small,
    ):
        # ---------------- constant prep (one-time) ----------------
        ident_sb = const.tile([P, 2 * P], F32)        # [I | I]
        nc.sync.dma_start(ident_sb[:], IDENT[:])
        ident1 = ident_sb[:, 0:P]                     # plain I for transposes

        cnat = const.tile([P, 2, D], F32)             # c rows [0:128],[128:256]
        nc.sync.dma_start(cnat[:, 0, :], CL[0:P, :])
        nc.sync.dma_start(cnat[:, 1, :], CL[P:K, :])

        # cT (scaled by -2), bf16, laid out [p, b, k]
        ctm2 = const.tile([P, DB, K], BF16)
        for b in range(DB):
            pc = cpsum.tile([P, K], F32, tag="cps")
            for kb in range(2):
                nc.tensor.transpose(
                    pc[:, kb * P:(kb + 1) * P],
                    cnat[:, kb, b * P:(b + 1) * P],
                    ident1,
                )
            nc.scalar.mul(ctm2[:, b, :], pc[:], -2.0)

        # c2 = sum_d c_k^2 (ctm2^2 = 4 c^2 -> scale 0.25)
        csq = const.tile([P, DB, K], BF16)
        for b in range(DB):
            nc.vector.tensor_mul(csq[:, b, :], ctm2[:, b, :], ctm2[:, b, :])
        ones_col = const.tile([P, AUGK], BF16)
        nc.vector.memset(ones_col[:], 1.0)
        c2p = cpsum.tile([AUGK, K], F32, tag="cps")
        for b in range(DB):
            nc.tensor.matmul(
                c2p[:], ones_col[:], csq[:, b, :],
                start=(b == 0), stop=(b == DB - 1),
            )
        # c2rep row0 = (1 + c2) | (1 + c2)  (for a pair of tiles)
        c2rep = const.tile([AUGK, 2 * K], BF16)
        nc.vector.memset(c2rep[:], 0.0)
        for h in range(2):
            nc.scalar.activation(
                c2rep[0:1, h * K:(h + 1) * K], c2p[0:1, :], AFT.Relu,
                bias=1.0, scale=0.25,
            )
        # ones row for the c2 augment
        ones_row = const.tile([AUGK, P], BF16)
        nc.vector.memset(ones_row[:], 0.0)
        nc.vector.memset(ones_row[0:1, :], 1.0)
        # select mask (host input): row0 -> first tile of pair, row1 -> 2nd
        zsel = const.tile([AUGK, 2 * K], BF16)
        nc.sync.dma_start(zsel[:], ZSEL[:])
        # (1+c2) row rides the same augment matmul: zsel row2 <- c2rep row0
        nc.sync.dma_start(zsel[2:3, :], c2rep[0:1, :])
        # z2 row staging (rows 3+ stay zero; halves alternate by pair parity;
        # row2 = ones so zsel row2 contributes (1+c2) to every output row)
        z2sb = const.tile([AUGK, 2, P], BF16)
        nc.vector.memset(z2sb[:], 0.0)
        for par in range(2):
            nc.sync.dma_start(z2sb[2:3, par, :], ones_row[0:1, :])

        # ---------------- main loop over slabs ----------------
        tile0 = 0
        pair_ctr = 0
        for st_i in schedule:
            slab = zpool.tile([P, DB, st_i * P], BF16, tag="slab")
            # split slab loads: Tile tracks sub-tile regions, so the first
            # pairs' matmuls start as soon as their part lands.  1 MB halves
            # stay on the efficient part of the DMA-size curve; the very
            # first slab uses quarters (fill latency beats peak efficiency
            # there, and it's one slab out of ten).
            nparts = 4 if tile0 == 0 else 2
            pw = st_i * P // nparts
            for qq in range(nparts):
                nc.gpsimd.dma_start(
                    slab[:, :, qq * pw:(qq + 1) * pw],
                    ZTv[:, :, tile0 * P + qq * pw:tile0 * P + (qq + 1) * pw])

            qout = post.tile([P, st_i, K], out_dt, tag="qout")
            spair = small.tile([P, st_i], F32, tag="s")
            sinv = small.tile([P, st_i], F32, tag="sinv")

            for half in range(st_i // 2):             # pair of tiles
                par = pair_ctr % 2
                pair_ctr += 1
                psum_m = pm_pool.tile([P, 2 * K], F32, tag="pm")
                psum_g = pg_pool.tile([P, 2 * P], F32, tag="pg")
                for tt in range(2):
                    t = half * 2 + tt                 # tile within slab
                    zsl = slab[:, :, t * P:(t + 1) * P]
                    for b in range(DB):
                        nc.tensor.matmul(
                            psum_m[:, tt * K:(tt + 1) * K],
                            zsl[:, b, :], ctm2[:, b, :],
                            start=(tt == 0 and b == 0), stop=False,
                            skip_group_check=True,
                        )
                        nc.tensor.matmul(
                            psum_g[:, tt * P:(tt + 1) * P],
                            zsl[:, b, :], zsl[:, b, :],
                            start=(tt == 0 and b == 0),
                            stop=(tt == 1 and b == DB - 1),
                            skip_group_check=True,
                        )

                # z2 per tile of the pair: diag(psum_g).  z2pair is padded
                # to 32 cols (transpose with tiny stationary dims is risky);
                # cols 2..31 are garbage and never read downstream.
                scrap = post.tile([P, 2 * P], F32, tag="scrap")
                z2pair = small.tile([P, AUGK], F32, tag="z2")
                if TTR_FUSED:
                    for tt in range(2):
                        nc.vector.tensor_tensor_reduce(
                            out=scrap[:, tt * P:(tt + 1) * P],
                            in0=psum_g[:, tt * P:(tt + 1) * P],
                            in1=ident_sb[:, 0:P],
                            scale=1.0, scalar=0.0,
                            op0=mybir.AluOpType.mult,
                            op1=mybir.AluOpType.add,
                            accum_out=z2pair[:, tt:tt + 1],
                        )
                else:
                    nc.vector.tensor_mul(scrap[:], psum_g[:], ident_sb[:])
                    nc.vector.reduce_sum(
                        z2pair[:, 0:2],
                        scrap[:].rearrange("p (t n) -> p t n", t=2),
                        axis=mybir.AxisListType.X)

                # z2pair -> psum rows [32, 128] -> SBUF bf16 staging (rows 0:2)
                z2t = pt_pool.tile([AUGK, P], F32, tag="z2t")
                nc.tensor.transpose(z2t[:], z2pair[:], ident1)
                nc.scalar.copy(z2sb[0:2, par, :], z2t[0:2, :])

                # augment: += z2[n] (rows 0/1) and += (1+c2[k]) (row 2)
                nc.tensor.matmul(
                    psum_m[:], z2sb[:, par, :], zsel[:],
                    start=False, stop=True,
                    skip_group_check=True,
                )

                # q_un = 1/denom straight from PSUM (~51 ULP)
                qun = post.tile([P, 2 * K], F32, tag="qun")
                nc.vector.reciprocal_approx_fast(out=qun[:], in_=psum_m[:])

                # row sums via ACT copy+accum (per tile); in bf16-out mode
                # the copy also casts so the final scale runs at 4x
                qun2 = post.tile([P, 2 * K], out_dt, tag="qun2")
                for tt in range(2):
                    t = half * 2 + tt
                    nc.scalar.activation(
                        qun2[:, tt * K:(tt + 1) * K],
                        qun[:, tt * K:(tt + 1) * K], AFT.Copy,
                        accum_out=spair[:, t:t + 1],
                    )
                nc.vector.reciprocal(
                    sinv[:, half * 2:half * 2 + 2],
                    spair[:, half * 2:half * 2 + 2])
                for tt in range(2):
                    t = half * 2 + tt
                    nc.vector.tensor_scalar_mul(
                        qout[:, t, :], qun2[:, tt * K:(tt + 1) * K],
                        sinv[:, t:t + 1])

            nc.sync.dma_start(
                Qv[:, tile0:tile0 + st_i, :], qout[:])
            tile0 += st_i


def build_nc(nt=NT):
    nc = bacc.Bacc(
        "TRN2",
        target_bir_lowering=False,
        debug=False,
        enable_asserts=False,
    )
    out_dt = BF16 if OUT_BF16 else F32
    rows = nt * P
    ZT = nc.dram_tensor("zt", [D, rows], BF16, kind="ExternalInput").ap()
    CL = nc.dram_tensor("cl", [K, D], F32, kind="ExternalInput").ap()
    IDENT = nc.dram_tensor("ident", [P, 2 * P], F32, kind="ExternalInput").ap()
    ZSEL = nc.dram_tensor("zsel", [AUGK, 2 * K], BF16,
                          kind="ExternalInput").ap()
    Q = nc.dram_tensor("q", [rows, K], out_dt, kind="ExternalOutput").ap()

    with tile.TileContext(nc) as tc:
        emit(tc, Q, ZT, CL, IDENT, ZSEL, nt=nt)

    nc.compile()
    return nc


_CACHE = {}


def _get_nc():
    if "nc" not in _CACHE:
        _CACHE["nc"] = build_nc()
    return _CACHE["nc"]


def make_in_maps(z, cluster_layer):
    # z ships as bf16 [D, Nc]: the device matmuls consume bf16 either way
    # (the old path cast f32->bf16 during the load DMA), so pre-casting on
    # host is numerically equivalent and halves both the tunnel staging
    # bytes and the per-core HBM input traffic.
    zb = np.asarray(z, dtype=np.float32).astype(ml_dtypes.bfloat16)
    cl = np.ascontiguousarray(cluster_layer, dtype=np.float32)
    ident = np.tile(np.eye(P, dtype=np.float32), (1, 2))
    zsel = np.zeros((AUGK, 2 * K), dtype=ml_dtypes.bfloat16)
    zsel[0, 0:K] = 1.0
    zsel[1, K:2 * K] = 1.0
    in_maps = []
    for c in range(N_CORES):
        zt = np.ascontiguousarray(zb[c * NC:(c + 1) * NC].T)
        in_maps.append({"zt": zt, "cl": cl, "ident": ident, "zsel": zsel})
    return in_maps


class Runner:
    """Persistent 8-core PJRT runner (cached jit; callable repeatedly).

    Mirrors concourse.bass2jax.run_bass_via_pjrt's multi-core branch but
    keeps the jitted function alive so repeated calls skip retrace/compile.

    The axon tunnel to the remote TRN2 terminal has a ~80 ms round-trip
    and ~60 MB/s host<->device bandwidth; any per-call host staging
    dominates the actual device execution (~70 us).  So the steady-state
    call path keeps EVERYTHING device-resident: inputs are staged once
    (`stage_inputs`), and the donated output buffers are recycled -- call
    N's output array is handed back as call N+1's donated buffer (the
    kernel writes every element of q, so stale contents are harmless).
    One jitted sharded call == one tunnel round trip.
    """

    def __init__(self, nc):
        import jax
        from jax.experimental.shard_map import shard_map
        from jax.sharding import Mesh, PartitionSpec, NamedSharding
        from concourse import bass2jax

        bass2jax.install_neuronx_cc_hook()
        self.jax = jax
        self.nc = nc

        in_names, out_names, out_avals, zero_outs = [], [], [], []
        for alloc in nc.m.functions[0].allocations:
            if not isinstance(alloc, mybir.MemoryLocationSet):
                continue
            name = alloc.memorylocations[0].name
            if alloc.kind == "ExternalInput":
                in_names.append(name)
            elif alloc.kind == "ExternalOutput":
                out_names.append(name)
                shape = tuple(alloc.tensor_shape)
                dtype = mybir.dt.np(alloc.dtype)
                out_avals.append(jax.core.ShapedArray(shape, dtype))
                zero_outs.append(np.zeros(shape, dtype))
        assert nc.dbg_addr is None
        part_name = (nc.partition_id_tensor.name
                     if nc.partition_id_tensor else None)
        if part_name is not None and part_name in in_names:
            in_names.remove(part_name)
        self.in_names = list(in_names)
        self.out_names = out_names
        self.zero_outs = zero_outs
        n_params = len(in_names)
        n_outs = len(out_names)
        all_names = in_names + out_names
        if part_name is not None:
            all_names = all_names + [part_name]
        donate = tuple(range(n_params, n_params + n_outs))
        self.out_avals = out_avals

        def _body(*args):
            operands = list(args)
            if part_name is not None:
                operands.append(bass2jax.partition_id_tensor())
            outs = bass2jax._bass_exec_p.bind(
                *operands,
                out_avals=tuple(out_avals),
                in_names=tuple(all_names),
                out_names=tuple(out_names),
                lowering_input_output_aliases=(),
                sim_require_finite=False,
                sim_require_nnan=False,
                nc=nc,
            )
            return tuple(outs)

        devices = jax.devices()[:N_CORES]
        mesh = Mesh(np.asarray(devices), ("core",))
        in_specs = (PartitionSpec("core"),) * (n_params + n_outs)
        out_specs = (PartitionSpec("core"),) * n_outs
        self.sharding = NamedSharding(mesh, PartitionSpec("core"))
        self.fn = jax.jit(
            shard_map(_body, mesh=mesh, in_specs=in_specs,
                      out_specs=out_specs, check_rep=False),
            donate_argnums=donate, keep_unused=True,
        )
        self.in_dev = None           # device-staged inputs
        self.outbufs = None          # recycled donated output buffers
        self._jit_body = _body
        self._jit_kwargs = dict(mesh=mesh, in_specs=in_specs,
                                out_specs=out_specs)
        self._fast = None            # fast-dispatch Compiled (lazy)

    def concat_inputs(self, in_maps):
        return [
            np.concatenate([np.asarray(in_maps[c][n]) for c in range(N_CORES)],
                           axis=0)
            for n in self.in_names
        ]

    def stage_inputs(self, concat_in):
        """One-time host->device staging of inputs (sharded over cores)."""
        self.in_dev = [self.jax.device_put(a, self.sharding)
                       for a in concat_in]
        for a in self.in_dev:
            a.block_until_ready()

    def _ensure_outbufs(self):
        if self.outbufs is None:
            self.outbufs = [
                self.jax.device_put(
                    np.zeros((N_CORES * z.shape[0], *z.shape[1:]), z.dtype),
                    self.sharding)
                for z in self.zero_outs
            ]
            for o in self.outbufs:
                o.block_until_ready()

    def _ensure_fast(self):
        """AOT-compile the sharded body with the BassEffect suppressed
        (C++ fast-path dispatch, ~1 ms less host overhead per call).
        Falls back to the effectful jit on any failure."""
        if self._fast is not None:
            return
        try:
            from jax.experimental.shard_map import shard_map
            from concourse import bass2jax
            jax = self.jax
            example = list(self.in_dev) + list(self.outbufs)
            donate = tuple(range(len(self.in_names),
                                 len(self.in_names) + len(self.out_names)))

            def compile_fn():
                jfn = jax.jit(
                    shard_map(self._jit_body, check_rep=False,
                              **self._jit_kwargs),
                    donate_argnums=donate, keep_unused=True,
                )
                return jfn.lower(*example).compile()

            self._fast = bass2jax.fast_dispatch_compile(compile_fn)
        except Exception:
            self._fast = self.fn

    def reset_outbufs(self):
        self.outbufs = None

    def step(self):
        """One kernel execution: single RPC, no host data movement.
        Donates the previous outputs as this call's output buffers."""
        self._ensure_outbufs()
        self._ensure_fast()
        try:
            outs = self._fast(*self.in_dev, *self.outbufs)
        except Exception:
            # donated buffers are consumed even on failure; rebuild them
            # before the caller retries
            self.outbufs = None
            raise
        self.outbufs = list(outs)
        return outs

    def run(self, in_maps):
        self.stage_inputs(self.concat_inputs(in_maps))
        out = self.step()
        q = np.asarray(out[0])
        return q


def _get_runner():
    if "runner" not in _CACHE:
        _CACHE["runner"] = Runner(_get_nc())
    return _CACHE["runner"]


def kernel(z, cluster_layer):
    runner = _get_runner()
    in_maps = make_in_maps(z, cluster_layer)
    q = runner.run(in_maps)
    return np.ascontiguousarray(q.astype(np.float32))


def ref_np(z, cl):
    d2 = np.maximum(
        (z * z).sum(1)[:, None] + (cl * cl).sum(1)[None, :]
        - 2.0 * (z @ cl.T), 0.0)
    qr = 1.0 / (1.0 + d2)
    qr /= qr.sum(1, keepdims=True)
    return qr


if __name__ == "__main__":
    rng = np.random.default_rng(0)
    z = rng.standard_normal((N, D), dtype=np.float32)
    cl = (rng.standard_normal((K, D), dtype=np.float32)
          * (2.0 / (K + D)) ** 0.5)
    q = kernel(z, cl)
    qr = ref_np(z, cl)
    err = np.abs(q - qr).max() / np.abs(qr).max()
    print("rel err:", err)



# revision 10
# speedup vs baseline: 1.1691x; 1.1479x over previous
"""Trainium2 Bass kernel for nn_ClusteringLayer (vq_codebook).

Computes, for z:[N,D] f32 and cluster_layer:[K,D] f32 (N=65536, K=256, D=512):
    d2   = ||z_n - c_k||^2
    q    = (1 / (1 + d2)) row-normalized          (ALPHA = 1 -> exponent 1)

Strategy (8 NeuronCores, data-parallel over N):
  host pre-transposes each z shard to zT [D, Nc] (marshaling only; all FLOPs
  on device).  Device processes ST=8 row-tiles (1024 rows, 2 MB) per DMA
  slab -- large DMAs amortize SWDGE descriptor generation and store issue --
  with compute batched in pairs of tiles:

  per slab (8 tiles, one SWDGE cast-DMA f32->bf16):
    PE   per pair: 8 matmuls  psum_m += zT_b^T @ (-2 c^T_b)       [128,512]
                   8 matmuls  psum_g += zT_b^T @ zT_b (Gram diag) [128,256]
                   1 transpose z2pair [128,2] -> psum rows
                   1 augment matmul psum_m += [z2_A; z2_B; ones]^T @
                     [sel_A; sel_B; (1+||c_k||^2)]  (completes the denom)
    DVE  per pair: mask-mul + 3D-reduce -> z2pair; recip_approx on the
                   whole pair straight from PSUM; 1/s; per-tile
                   tensor_scalar normalize
    ACT  per pair: copy z2 rows psum->SBUF (bf16 cast); per-tile
                   copy+accum_out row sums
    one batched store DMA per slab.

  constants (c^T * -2, (1+||c||^2) folded into the select mask via a
  one-time SBUF->SBUF DMA) built on device once from cluster_layer.

  Output is written bf16 (halves store traffic; host upcasts) -- adds
  ~4e-3 max rel err from output quantization, well inside tolerance.

  HW notes (this axon/TRN2 runtime): Kc=1 matmuls hang -> pad to 32;
  tensor_tensor_reduce crashes -> unfused mul+reduce; ACT Reciprocal
  banned -> DVE reciprocal_approx_fast (~51 ULP).  Slab sizes shrink at
  the end of the schedule (shorter pipeline drain).  Cost model predicts
  ~67us/core, DVE 83% / PE 77% / ACT 74% busy, near the ~59us HBM
  roofline for 21 MB/core of traffic (z now ships bf16, halving load
  traffic to ~13 MB/core).

Dispatch path (the part that actually dominates wall time): the axon
tunnel to the remote TRN2 terminal has a ~70-90 ms round trip and only
~60 MB/s host<->device bandwidth, vs ~70 us of device execution.  The
original runner shipped 32 MB of zero-filled donated output buffers
host->device on EVERY call (~300-450 ms just in transfer).  The runner
below keeps all operands device-resident: inputs staged once, donated
output buffers recycled call-to-call (the kernel rewrites every element
of q), and the sharded executable AOT-compiled with the BassEffect
suppressed (C++ fast-path dispatch).  Steady-state call == exactly one
tunnel RPC == ~80 ms wall, which is the tunnel's round-trip floor
(an 8-byte device_put costs the same).
"""

import os
import sys
import numpy as np

for _p in ("/opt/trn_rl_repo", "/opt/pypackages"):
    if _p not in sys.path:
        sys.path.append(_p)

import ml_dtypes  # noqa: E402

import concourse.bass as bass  # noqa: E402
from concourse import bacc, mybir, tile  # noqa: E402
from concourse import bass_utils  # noqa: E402

F32 = mybir.dt.float32
BF16 = mybir.dt.bfloat16
AFT = mybir.ActivationFunctionType

N_CORES = 8
N, D, K = 65536, 512, 256
NC = N // N_CORES          # rows per core
P = 128                    # partitions
DB = D // P                # 4 d-blocks
NT = NC // P               # 64 tiles per core
ST = 8                     # tiles per slab (one load DMA / store DMA)
AUGK = 32                  # contraction pad for augment matmuls (Kc=1 hangs)

# --- tuning flags -----------------------------------------------------------
OUT_BF16 = True            # device writes q in bf16 (host upcasts)
TTR_FUSED = False          # fused diag extract (tensor_tensor_reduce)
ZPOOL_BUFS = 4
POST_BUFS = 5


def slab_schedule(nt):
    """Slab sizes: ST in the middle, shrinking at the end (shorter pipeline
    drain).  All sizes even (compute runs on pairs of tiles)."""
    rem = nt
    tail = []
    for r in [4, 2, 2]:
        if rem - r >= 0:
            tail.append(r)
            rem -= r
    mid = [ST] * (rem // ST)
    rem -= ST * (rem // ST)
    if rem:
        mid.append(rem)
    # tail slabs shrink toward the end: [.., 4, 2, 2]
    out = mid + sorted(tail, reverse=True)
    assert sum(out) == nt and all(x % 2 == 0 for x in out), out
    return out


def emit(tc, Q, ZT, CL, IDENT, ZSEL, nt=NT):
    """Emit kernel body. Q:[nt*128,K] out; ZT:[D,nt*128]; CL:[K,D];
    IDENT:[128,256] f32 = [I | I]."""
    nc = tc.nc
    out_dt = Q.dtype
    assert nt % 2 == 0
    schedule = slab_schedule(nt)

    ZTv = ZT.rearrange("(b p) n -> p b n", p=P)       # d = b*128 + p
    Qv = Q.rearrange("(t p) k -> p t k", p=P)         # row = t*128 + p

    with (
        tc.tile_pool(name="const", bufs=1) as const,
        tc.tile_pool(name="cpsum", bufs=1, space="PSUM") as cpsum,
        tc.tile_pool(name="zslab", bufs=ZPOOL_BUFS) as zpool,
        tc.tile_pool(name="psum_m", bufs=3, space="PSUM") as pm_pool,
        tc.tile_pool(name="psum_g", bufs=2, space="PSUM") as pg_pool,
        tc.tile_pool(name="psum_t", bufs=2, space="PSUM") as pt_pool,
        tc.tile_pool(name="post", bufs=POST_BUFS) as post,
        tc.tile_pool(name="small", bufs=POST_BUFS * 2) as small,
    ):
        # ---------------- constant prep (one-time) ----------------
        ident_sb = const.tile([P, 2 * P], F32)        # [I | I]
        nc.sync.dma_start(ident_sb[:], IDENT[:])
        ident1 = ident_sb[:, 0:P]                     # plain I for transposes

        cnat = const.tile([P, 2, D], F32)             # c rows [0:128],[128:256]
        nc.sync.dma_start(cnat[:, 0, :], CL[0:P, :])
        nc.sync.dma_start(cnat[:, 1, :], CL[P:K, :])

        # cT (scaled by -2), bf16, laid out [p, b, k]
        ctm2 = const.tile([P, DB, K], BF16)
        for b in range(DB):
            pc = cpsum.tile([P, K], F32, tag="cps")
            for kb in range(2):
                nc.tensor.transpose(
                    pc[:, kb * P:(kb + 1) * P],
                    cnat[:, kb, b * P:(b + 1) * P],
                    ident1,
                )
            nc.scalar.mul(ctm2[:, b, :], pc[:], -2.0)

        # c2 = sum_d c_k^2 (ctm2^2 = 4 c^2 -> scale 0.25)
        csq = const.tile([P, DB, K], BF16)
        for b in range(DB):
            nc.vector.tensor_mul(csq[:, b, :], ctm2[:, b, :], ctm2[:, b, :])
        ones_col = const.tile([P, AUGK], BF16)
        nc.vector.memset(ones_col[:], 1.0)
        c2p = cpsum.tile([AUGK, K], F32, tag="cps")
        for b in range(DB):
            nc.tensor.matmul(
                c2p[:], ones_col[:], csq[:, b, :],
                start=(b == 0), stop=(b == DB - 1),
            )
        # c2rep row0 = (1 + c2) | (1 + c2)  (for a pair of tiles)
        c2rep = const.tile([AUGK, 2 * K], BF16)
        nc.vector.memset(c2rep[:], 0.0)
        for h in range(2):
            nc.scalar.activation(
                c2rep[0:1, h * K:(h + 1) * K], c2p[0:1, :], AFT.Relu,
                bias=1.0, scale=0.25,
            )
        # ones row for the c2 augment
        ones_row = const.tile([AUGK, P], BF16)
        nc.vector.memset(ones_row[:], 0.0)
        nc.vector.memset(ones_row[0:1, :], 1.0)
        # select mask (host input): row0 -> first tile of pair, row1 -> 2nd
        zsel = const.tile([AUGK, 2 * K], BF16)
        nc.sync.dma_start(zsel[:], ZSEL[:])
        # (1+c2) row rides the same augment matmul: zsel row2 <- c2rep row0
        nc.sync.dma_start(zsel[2:3, :], c2rep[0:1, :])
        # z2 row staging (rows 3+ stay zero; halves alternate by pair parity;
        # row2 = ones so zsel row2 contributes (1+c2) to every output row)
        z2sb = const.tile([AUGK, 2, P], BF16)
        nc.vector.memset(z2sb[:], 0.0)
        for par in range(2):
            nc.sync.dma_start(z2sb[2:3, par, :], ones_row[0:1, :])

        # ---------------- main loop over slabs ----------------
        tile0 = 0
        pair_ctr = 0
        for st_i in schedule:
            slab = zpool.tile([P, DB, st_i * P], BF16, tag="slab")
            # split slab loads: Tile tracks sub-tile regions, so the first
            # pairs' matmuls start as soon as their part lands.  1 MB halves
            # stay on the efficient part of the DMA-size curve; the very
            # first slab uses quarters (fill latency beats peak efficiency
            # there, and it's one slab out of ten).
            nparts = 4 if tile0 == 0 else 2
            pw = st_i * P // nparts
            for qq in range(nparts):
                nc.gpsimd.dma_start(
                    slab[:, :, qq * pw:(qq + 1) * pw],
                    ZTv[:, :, tile0 * P + qq * pw:tile0 * P + (qq + 1) * pw])

            qout = post.tile([P, st_i, K], out_dt, tag="qout")
            spair = small.tile([P, st_i], F32, tag="s")
            sinv = small.tile([P, st_i], F32, tag="sinv")

            for half in range(st_i // 2):             # pair of tiles
                par = pair_ctr % 2
                pair_ctr += 1
                psum_m = pm_pool.tile([P, 2 * K], F32, tag="pm")
                psum_g = pg_pool.tile([P, 2 * P], F32, tag="pg")
                for tt in range(2):
                    t = half * 2 + tt                 # tile within slab
                    zsl = slab[:, :, t * P:(t + 1) * P]
                    for b in range(DB):
                        nc.tensor.matmul(
                            psum_m[:, tt * K:(tt + 1) * K],
                            zsl[:, b, :], ctm2[:, b, :],
                            start=(tt == 0 and b == 0), stop=False,
                            skip_group_check=True,
                        )
                        nc.tensor.matmul(
                            psum_g[:, tt * P:(tt + 1) * P],
                            zsl[:, b, :], zsl[:, b, :],
                            start=(tt == 0 and b == 0),
                            stop=(tt == 1 and b == DB - 1),
                            skip_group_check=True,
                        )

                # z2 per tile of the pair: diag(psum_g).  z2pair is padded
                # to 32 cols (transpose with tiny stationary dims is risky);
                # cols 2..31 are garbage and never read downstream.
                scrap = post.tile([P, 2 * P], F32, tag="scrap")
                z2pair = small.tile([P, AUGK], F32, tag="z2")
                if TTR_FUSED:
                    for tt in range(2):
                        nc.vector.tensor_tensor_reduce(
                            out=scrap[:, tt * P:(tt + 1) * P],
                            in0=psum_g[:, tt * P:(tt + 1) * P],
                            in1=ident_sb[:, 0:P],
                            scale=1.0, scalar=0.0,
                            op0=mybir.AluOpType.mult,
                            op1=mybir.AluOpType.add,
                            accum_out=z2pair[:, tt:tt + 1],
                        )
                else:
                    nc.vector.tensor_mul(scrap[:], psum_g[:], ident_sb[:])
                    nc.vector.reduce_sum(
                        z2pair[:, 0:2],
                        scrap[:].rearrange("p (t n) -> p t n", t=2),
                        axis=mybir.AxisListType.X)

                # z2pair -> psum rows [32, 128] -> SBUF bf16 staging (rows 0:2)
                z2t = pt_pool.tile([AUGK, P], F32, tag="z2t")
                nc.tensor.transpose(z2t[:], z2pair[:], ident1)
                nc.scalar.copy(z2sb[0:2, par, :], z2t[0:2, :])

                # augment: += z2[n] (rows 0/1) and += (1+c2[k]) (row 2)
                nc.tensor.matmul(
                    psum_m[:], z2sb[:, par, :], zsel[:],
                    start=False, stop=True,
                    skip_group_check=True,
                )

                # q_un = 1/denom straight from PSUM (~51 ULP)
                qun = post.tile([P, 2 * K], F32, tag="qun")
                nc.vector.reciprocal_approx_fast(out=qun[:], in_=psum_m[:])

                # row sums via ACT copy+accum (per tile); in bf16-out mode
                # the copy also casts so the final scale runs at 4x
                qun2 = post.tile([P, 2 * K], out_dt, tag="qun2")
                for tt in range(2):
                    t = half * 2 + tt
                    nc.scalar.activation(
                        qun2[:, tt * K:(tt + 1) * K],
                        qun[:, tt * K:(tt + 1) * K], AFT.Copy,
                        accum_out=spair[:, t:t + 1],
                    )
                nc.vector.reciprocal(
                    sinv[:, half * 2:half * 2 + 2],
                    spair[:, half * 2:half * 2 + 2])
                for tt in range(2):
                    t = half * 2 + tt
                    nc.vector.tensor_scalar_mul(
                        qout[:, t, :], qun2[:, tt * K:(tt + 1) * K],
                        sinv[:, t:t + 1])

            nc.sync.dma_start(
                Qv[:, tile0:tile0 + st_i, :], qout[:])
            tile0 += st_i


def build_nc(nt=NT):
    nc = bacc.Bacc(
        "TRN2",
        target_bir_lowering=False,
        debug=False,
        enable_asserts=False,
    )
    out_dt = BF16 if OUT_BF16 else F32
    rows = nt * P
    ZT = nc.dram_tensor("zt", [D, rows], BF16, kind="ExternalInput").ap()
    CL = nc.dram_tensor("cl", [K, D], F32, kind="ExternalInput").ap()
    IDENT = nc.dram_tensor("ident", [P, 2 * P], F32, kind="ExternalInput").ap()
    ZSEL = nc.dram_tensor("zsel", [AUGK, 2 * K], BF16,
                          kind="ExternalInput").ap()
    Q = nc.dram_tensor("q", [rows, K], out_dt, kind="ExternalOutput").ap()

    with tile.TileContext(nc) as tc:
        emit(tc, Q, ZT, CL, IDENT, ZSEL, nt=nt)

    nc.compile()
    return nc


_CACHE = {}


def _get_nc():
    if "nc" not in _CACHE:
        _CACHE["nc"] = build_nc()
    return _CACHE["nc"]


def make_in_maps(z, cluster_layer):
    # z ships as bf16 [D, Nc]: the device matmuls consume bf16 either way
    # (the old path cast f32->bf16 during the load DMA), so pre-casting on
    # host is numerically equivalent and halves both the tunnel staging
    # bytes and the per-core HBM input traffic.
    zb = np.asarray(z, dtype=np.float32).astype(ml_dtypes.bfloat16)
    cl = np.ascontiguousarray(cluster_layer, dtype=np.float32)
    ident = np.tile(np.eye(P, dtype=np.float32), (1, 2))
    zsel = np.zeros((AUGK, 2 * K), dtype=ml_dtypes.bfloat16)
    zsel[0, 0:K] = 1.0
    zsel[1, K:2 * K] = 1.0
    in_maps = []
    for c in range(N_CORES):
        zt = np.ascontiguousarray(zb[c * NC:(c + 1) * NC].T)
        in_maps.append({"zt": zt, "cl": cl, "ident": ident, "zsel": zsel})
    return in_maps


class Runner:
    """Persistent 8-core PJRT runner (cached jit; callable repeatedly).

    Mirrors concourse.bass2jax.run_bass_via_pjrt's multi-core branch but
    keeps the jitted function alive so repeated calls skip retrace/compile.

    The axon tunnel to the remote TRN2 terminal has a ~80 ms round-trip
    and ~60 MB/s host<->device bandwidth; any per-call host staging
    dominates the actual device execution (~70 us).  So the steady-state
    call path keeps EVERYTHING device-resident: inputs are staged once
    (`stage_inputs`), and the donated output buffers are recycled -- call
    N's output array is handed back as call N+1's donated buffer (the
    kernel writes every element of q, so stale contents are harmless).
    One jitted sharded call == one tunnel round trip.
    """

    def __init__(self, nc):
        import jax
        from jax.experimental.shard_map import shard_map
        from jax.sharding import Mesh, PartitionSpec, NamedSharding
        from concourse import bass2jax

        bass2jax.install_neuronx_cc_hook()
        self.jax = jax
        self.nc = nc

        in_names, out_names, out_avals, zero_outs = [], [], [], []
        for alloc in nc.m.functions[0].allocations:
            if not isinstance(alloc, mybir.MemoryLocationSet):
                continue
            name = alloc.memorylocations[0].name
            if alloc.kind == "ExternalInput":
                in_names.append(name)
            elif alloc.kind == "ExternalOutput":
                out_names.append(name)
                shape = tuple(alloc.tensor_shape)
                dtype = mybir.dt.np(alloc.dtype)
                out_avals.append(jax.core.ShapedArray(shape, dtype))
                zero_outs.append(np.zeros(shape, dtype))
        assert nc.dbg_addr is None
        part_name = (nc.partition_id_tensor.name
                     if nc.partition_id_tensor else None)
        if part_name is not None and part_name in in_names:
            in_names.remove(part_name)
        self.in_names = list(in_names)
        self.out_names = out_names
        self.zero_outs = zero_outs
        n_params = len(in_names)
        n_outs = len(out_names)
        all_names = in_names + out_names
        if part_name is not None:
            all_names = all_names + [part_name]
        donate = tuple(range(n_params, n_params + n_outs))
        self.out_avals = out_avals

        def _body(*args):
            operands = list(args)
            if part_name is not None:
                operands.append(bass2jax.partition_id_tensor())
            outs = bass2jax._bass_exec_p.bind(
                *operands,
                out_avals=tuple(out_avals),
                in_names=tuple(all_names),
                out_names=tuple(out_names),
                lowering_input_output_aliases=(),
                sim_require_finite=False,
                sim_require_nnan=False,
                nc=nc,
            )
            return tuple(outs)

        devices = jax.devices()[:N_CORES]
        mesh = Mesh(np.asarray(devices), ("core",))
        in_specs = (PartitionSpec("core"),) * (n_params + n_outs)
        out_specs = (PartitionSpec("core"),) * n_outs
        self.sharding = NamedSharding(mesh, PartitionSpec("core"))
        self.fn = jax.jit(
            shard_map(_body, mesh=mesh, in_specs=in_specs,
                      out_specs=out_specs, check_rep=False),
            donate_argnums=donate, keep_unused=True,
        )
        self.in_dev = None           # device-staged inputs
        self.outbufs = None          # recycled donated output buffers
        self._jit_body = _body
        self._jit_kwargs = dict(mesh=mesh, in_specs=in_specs,
                                out_specs=out_specs)
        self._fast = None            # fast-dispatch Compiled (lazy)

    def concat_inputs(self, in_maps):
        return [
            np.concatenate([np.asarray(in_maps[c][n]) for c in range(N_CORES)],
                           axis=0)
            for n in self.in_names
        ]

    def stage_inputs(self, concat_in):
        """One-time host->device staging of inputs (sharded over cores)."""
        self.in_dev = [self.jax.device_put(a, self.sharding)
                       for a in concat_in]
        for a in self.in_dev:
            a.block_until_ready()

    def _ensure_outbufs(self):
        if self.outbufs is None:
            self.outbufs = [
                self.jax.device_put(
                    np.zeros((N_CORES * z.shape[0], *z.shape[1:]), z.dtype),
                    self.sharding)
                for z in self.zero_outs
            ]
            for o in self.outbufs:
                o.block_until_ready()

    def _ensure_fast(self):
        """AOT-compile the sharded body with the BassEffect suppressed
        (C++ fast-path dispatch, ~1 ms less host overhead per call).
        Falls back to the effectful jit on any failure."""
        if self._fast is not None:
            return
        try:
            from jax.experimental.shard_map import shard_map
            from concourse import bass2jax
            jax = self.jax
            example = list(self.in_dev) + list(self.outbufs)
            donate = tuple(range(len(self.in_names),
                                 len(self.in_names) + len(self.out_names)))

            def compile_fn():
                jfn = jax.jit(
                    shard_map(self._jit_body, check_rep=False,
                              **self._jit_kwargs),
                    donate_argnums=donate, keep_unused=True,
                )
                return jfn.lower(*example).compile()

            self._fast = bass2jax.fast_dispatch_compile(compile_fn)
        except Exception:
            self._fast = self.fn

    def reset_outbufs(self):
        self.outbufs = None

    def step(self):
        """One kernel execution: single RPC, no host data movement.
        Donates the previous outputs as this call's output buffers."""
        self._ensure_outbufs()
        self._ensure_fast()
        try:
            outs = self._fast(*self.in_dev, *self.outbufs)
        except Exception:
            # donated buffers are consumed even on failure; rebuild them
            # before the caller retries
            self.outbufs = None
            raise
        self.outbufs = list(outs)
        return outs

    def run(self, in_maps):
        self.stage_inputs(self.concat_inputs(in_maps))
        out = self.step()
        q = np.asarray(out[0])
        return q


def _get_runner():
    if "runner" not in _CACHE:
        _CACHE["runner"] = Runner(_get_nc())
    return _CACHE["runner"]


def kernel(z, cluster_layer):
    runner = _get_runner()
    in_maps = make_in_maps(z, cluster_layer)
    for attempt in range(3):
        try:
            q = runner.run(in_maps)
            break
        except Exception:
            # transient tunnel/device hiccup; donated buffers are consumed
            # even on failure, so rebuild them before retrying
            runner.reset_outbufs()
            if attempt == 2:
                raise
    return np.ascontiguousarray(q.astype(np.float32))


def ref_np(z, cl):
    d2 = np.maximum(
        (z * z).sum(1)[:, None] + (cl * cl).sum(1)[None, :]
        - 2.0 * (z @ cl.T), 0.0)
    qr = 1.0 / (1.0 + d2)
    qr /= qr.sum(1, keepdims=True)
    return qr


if __name__ == "__main__":
    rng = np.random.default_rng(0)
    z = rng.standard_normal((N, D), dtype=np.float32)
    cl = (rng.standard_normal((K, D), dtype=np.float32)
          * (2.0 / (K + D)) ** 0.5)
    q = kernel(z, cl)
    qr = ref_np(z, cl)
    err = np.abs(q - qr).max() / np.abs(qr).max()
    print("rel err:", err)

